# revision 1
# baseline (speedup 1.0000x reference)
"""AUGRU cell kernel for trn2, 8-core data-parallel.

Layout: transposed ("feature-major") — features on partitions, batch rows on
the free dim. Host transposes x/h_1/a into [256, B/8] per core and transposes
the output back. Weights W[K,M] are directly the stationary lhsT; biases are
per-partition in this layout and fold into ScalarE activation bias. The
attention scalar a (per-row) is broadcast across partitions by a stride-0
DMA from DRAM.

Matmuls run as float32r (raw fp32 bits DMA'd into f32r-typed tiles): full
bf16-rate on the PE at ~15x better precision than bf16 (measured 2.8e-4
scale-relative on a [256]x[256,256] gate).

Per row-chunk per feature-half m (all matmul pre-sums in PSUM):
  psum_u = Wu.T x + Uu.T h   -> u  = sigmoid(psum_u + bu)           [ScalarE]
  psum_r = Wr.T x + Ur.T h   -> r  = sigmoid(psum_r + br)           [ScalarE]
  psum_e = Uh.T h            -> t  = r * psum_e (or via bf16 evac)  [Sc/VecE]
  psum_g = Wh.T x            -> hp = psum_g + t ; hh = tanh(hp+bh)  [Vec/ScE]
  d = hh - h ; p = u * d ; q = a_bc * p                             [VectorE]
  outT += q   (SWDGE accumulating DMA onto outT, which the runner
               donates PREFILLED with hT — the DMA engine computes
               out = h + a*u*(hh - h))                              [GPSIMD DMA]

The output buffer prefill relies on run_spmd_prefill (donated buffers keep
their contents). `cbf` is a vestigial input (fed, unused).
"""
import numpy as np
from contextlib import ExitStack

import bass_rust
import ml_dtypes
import concourse.bass as bass
import concourse.mybir as mybir
import concourse.tile as tile
from concourse import library_config
from concourse.bass_utils import run_bass_kernel_spmd
import concourse.bass2jax as b2j

F32 = mybir.dt.float32
F32R = mybir.dt.float32r
BF16 = mybir.dt.bfloat16

B, D = 65536, 256
NCORES = 8
BL = B // NCORES          # rows per core
P = 128
N = 512                   # rows per chunk
NCHUNK = BL // N          # 16
CW = 6 * 512 + 6          # consts image cols: 6 weights [p,2,256] + 6 biases


def split_multi_waits(nc):
    """Walrus codegen allows at most one semaphore wait per instruction.
    Split extras onto preceding same-engine NoOps."""
    for fn in nc.m.functions:
        for bb in fn.blocks:
            out = []
            for inst in bb.instructions:
                si = inst.sync_info
                if si is not None and len(si.on_wait) > 1:
                    waits = list(si.on_wait)
                    for j, w in enumerate(waits[:-1]):
                        nop = bass_rust.InstNoOp(name=f"{inst.name}-sw{j}")
                        nop.engine = inst.engine
                        nop.sync_info = mybir.SyncInfo(on_wait=[w], on_update=[])
                        out.append(nop)
                    inst.sync_info = mybir.SyncInfo(
                        on_wait=[waits[-1]], on_update=list(si.on_update))
                out.append(inst)
            bb.instructions = out


def build(npass=1):
    nc = bass.Bass()
    c_d = nc.declare_dram_parameter("consts", [P, CW], F32R, isOutput=False)
    cbf_d = nc.declare_dram_parameter("cbf", [P, 256], BF16, isOutput=False)
    xT_d = nc.declare_dram_parameter("xT", [D, BL], F32R, isOutput=False)
    hT_d = nc.declare_dram_parameter("hT", [D, BL], F32R, isOutput=False)
    aT_d = nc.declare_dram_parameter("aT", [1, BL], BF16, isOutput=False)
    o_d = nc.declare_dram_parameter("outT", [D, BL], F32, isOutput=True)

    xT_ap = xT_d.ap().rearrange("(c p) n -> p c n", p=P)
    hT_ap = hT_d.ap().rearrange("(c p) n -> p c n", p=P)

    SIG = mybir.ActivationFunctionType.Sigmoid
    TANH = mybir.ActivationFunctionType.Tanh

    # chunk schedule: smaller first chunk (cheap startup) and final chunks
    # (short drain tail); 512-row chunks in the middle.
    chunks = [(0, 256), (256, 256)]
    chunks += [(512 + i * N, N) for i in range(NCHUNK - 2)]
    chunks += [((NCHUNK - 1) * N, 256), ((NCHUNK - 1) * N + 256, 256)]
    schedule = []
    for ip in range(npass):
        schedule += chunks

    with tile.TileContext(nc) as tc, ExitStack() as ctx:
        const = ctx.enter_context(tc.tile_pool(name="const", bufs=1))
        io = ctx.enter_context(tc.tile_pool(name="io", bufs=1))
        sm = ctx.enter_context(tc.tile_pool(name="sm", bufs=1))
        psum = ctx.enter_context(tc.tile_pool(name="psum", bufs=1, space="PSUM"))

        c_sb = const.tile([P, CW], F32R)
        nc.sync.dma_start(out=c_sb[:, 0:512], in_=c_d.ap()[:, 0:512])        # Wu
        warm_x = io.tile([P, 2, N], F32R, tag="x_t", bufs=4, name="x_t")
        nc.sync.dma_start(out=warm_x[:, :, 0:256], in_=xT_ap[:, :, 0:256])
        nc.sync.dma_start(out=c_sb[:, 512:1024], in_=c_d.ap()[:, 512:1024])  # Uu
        warm_h = io.tile([P, 2, N], F32R, tag="h_t", bufs=5, name="h_t")
        nc.sync.dma_start(out=warm_h[:, :, 0:256], in_=hT_ap[:, :, 0:256])
        nc.sync.dma_start(out=c_sb[:, 1024:2048], in_=c_d.ap()[:, 1024:2048])  # Wr,Ur
        nc.sync.dma_start(out=c_sb[:, 2048:CW], in_=c_d.ap()[:, 2048:CW])      # Wh,Uh,biases
        wu_all = c_sb[:, 0:3072].rearrange("p (g s c m) -> p g s c m", g=3, s=2, c=2)
        w_x = wu_all[:, :, 0]                          # [p, g, c, m*? ] x-side weights
        w_h = wu_all[:, :, 1]                          # h-side weights
        bias = c_sb[:, 3072:3078].bitcast(F32)        # [p, 6]: (bu,br,bh)x(m0,m1)
        W_U, W_R, W_H = 0, 1, 2
        U_U, U_R, U_H = 0, 1, 2


        for ci, (n0, NL) in enumerate(schedule):
            last = ci == len(schedule) - 1
            if ci == 0:
                x_t, h_t = warm_x, warm_h
            else:
                x_t = io.tile([P, 2, N], F32R, tag="x_t", bufs=4, name="x_t")
                nc.sync.dma_start(out=x_t[:, :, 0:NL], in_=xT_ap[:, :, n0:n0 + NL])
                h_t = io.tile([P, 2, N], F32R, tag="h_t", bufs=5, name="h_t")
                nc.sync.dma_start(out=h_t[:, :, 0:NL], in_=hT_ap[:, :, n0:n0 + NL])
            h_f = h_t.bitcast(F32)

            if ci == 0:
                # gate-complete emission matched to the startup supply order
                # [Wu, x, Uu, h, Wr+Ur, Wh+Uh]: each gate's psum group closes
                # fast, so PE streams right behind the DMA with no bank pileup
                ps0 = {}
                for gate, wg in (("u", W_U), ("r", W_R)):
                    for m0_ in range(2):
                        ms0 = slice(m0_ * P, (m0_ + 1) * P)
                        pst = psum.tile([P, N], F32, tag="ps", bufs=8, name="ps0")
                        nc.tensor.matmul(pst[:, 0:NL], w_x[:, wg, 0, ms0],
                                         x_t[:, 0, 0:NL], start=True, stop=False)
                        nc.tensor.matmul(pst[:, 0:NL], w_x[:, wg, 1, ms0],
                                         x_t[:, 1, 0:NL], start=False, stop=False)
                        nc.tensor.matmul(pst[:, 0:NL], w_h[:, wg, 0, ms0],
                                         h_t[:, 0, 0:NL], start=False, stop=False)
                        nc.tensor.matmul(pst[:, 0:NL], w_h[:, wg, 1, ms0],
                                         h_t[:, 1, 0:NL], start=False, stop=True)
                        ps0[(gate, m0_)] = pst
                for m0_ in range(2):
                    ms0 = slice(m0_ * P, (m0_ + 1) * P)
                    pst = psum.tile([P, N], F32, tag="ps", bufs=8, name="ps0")
                    nc.tensor.matmul(pst[:, 0:NL], w_x[:, W_H, 0, ms0],
                                     x_t[:, 0, 0:NL], start=True, stop=False)
                    nc.tensor.matmul(pst[:, 0:NL], w_x[:, W_H, 1, ms0],
                                     x_t[:, 1, 0:NL], start=False, stop=True)
                    ps0[("g", m0_)] = pst

            a_bc = sm.tile([P, N], BF16, tag="a_bc", bufs=3, name="a_bc")
            nc.sync.dma_start(out=a_bc[:, 0:NL],
                              in_=aT_d.ap()[0:1, n0:n0 + NL].to_broadcast((P, NL)))

            for m in range(2):
                ms = slice(m * P, (m + 1) * P)
                if ci == 0:
                    ps_u = ps0[("u", m)]
                else:
                    ps_u = psum.tile([P, N], F32, tag="ps", bufs=8, name="ps_u")
                    nc.tensor.matmul(ps_u[:, 0:NL], w_x[:, W_U, 0, ms], x_t[:, 0, 0:NL], start=True, stop=False)
                    nc.tensor.matmul(ps_u[:, 0:NL], w_x[:, W_U, 1, ms], x_t[:, 1, 0:NL], start=False, stop=False)
                    nc.tensor.matmul(ps_u[:, 0:NL], w_h[:, U_U, 0, ms], h_t[:, 0, 0:NL], start=False, stop=False)
                    nc.tensor.matmul(ps_u[:, 0:NL], w_h[:, U_U, 1, ms], h_t[:, 1, 0:NL], start=False, stop=True)
                u = sm.tile([P, N], BF16, tag=f"u{m}", bufs=2, name="u")
                nc.scalar.activation(u[:, 0:NL], ps_u[:, 0:NL], SIG, bias=bias[:, 0 + m:1 + m])

                if ci == 0:
                    ps_r = ps0[("r", m)]
                else:
                    ps_r = psum.tile([P, N], F32, tag="ps", bufs=8, name="ps_r")
                    nc.tensor.matmul(ps_r[:, 0:NL], w_x[:, W_R, 0, ms], x_t[:, 0, 0:NL], start=True, stop=False)
                    nc.tensor.matmul(ps_r[:, 0:NL], w_x[:, W_R, 1, ms], x_t[:, 1, 0:NL], start=False, stop=False)
                    nc.tensor.matmul(ps_r[:, 0:NL], w_h[:, U_R, 0, ms], h_t[:, 0, 0:NL], start=False, stop=False)
                    nc.tensor.matmul(ps_r[:, 0:NL], w_h[:, U_R, 1, ms], h_t[:, 1, 0:NL], start=False, stop=True)
                r = sm.tile([P, N], BF16, tag=f"r{m}", bufs=2, name="r")
                nc.scalar.activation(r[:, 0:NL], ps_r[:, 0:NL], SIG, bias=bias[:, 2 + m:3 + m])

                ps_e = psum.tile([P, N], F32, tag="ps", bufs=8, name="ps_e")
                nc.tensor.matmul(ps_e[:, 0:NL], w_h[:, U_H, 0, ms], h_t[:, 0, 0:NL], start=True, stop=False)
                nc.tensor.matmul(ps_e[:, 0:NL], w_h[:, U_H, 1, ms], h_t[:, 1, 0:NL], start=False, stop=True)
                t = sm.tile([P, N], BF16, tag=f"t{m}", bufs=3, name="t")
                if (2 * ci + m) % 3 == 0:
                    nc.vector.tensor_mul(out=t[:, 0:NL], in0=r[:, 0:NL], in1=ps_e[:, 0:NL])
                else:
                    e_sb = sm.tile([P, N], BF16, tag=f"e{m}", bufs=2, name="e_sb")
                    nc.scalar.activation(e_sb[:, 0:NL], ps_e[:, 0:NL],
                                         mybir.ActivationFunctionType.Copy)
                    nc.vector.tensor_mul(out=t[:, 0:NL], in0=r[:, 0:NL], in1=e_sb[:, 0:NL])

                if ci == 0:
                    ps_g = ps0[("g", m)]
                else:
                    ps_g = psum.tile([P, N], F32, tag="ps", bufs=8, name="ps_g")
                    nc.tensor.matmul(ps_g[:, 0:NL], w_x[:, W_H, 0, ms], x_t[:, 0, 0:NL], start=True, stop=False)
                    nc.tensor.matmul(ps_g[:, 0:NL], w_x[:, W_H, 1, ms], x_t[:, 1, 0:NL], start=False, stop=True)
                hp = sm.tile([P, N], F32, tag=f"hp{m}", bufs=3, name="hp")
                nc.vector.tensor_add(out=hp[:, 0:NL], in0=ps_g[:, 0:NL], in1=t[:, 0:NL])
                hh = sm.tile([P, N], BF16, tag=f"hh{m}", bufs=3, name="hh")
                nc.scalar.activation(hh[:, 0:NL], hp[:, 0:NL], TANH, bias=bias[:, 4 + m:5 + m])

                d = sm.tile([P, N], BF16, tag=f"d{m}", bufs=3, name="d")
                nc.vector.tensor_sub(out=d[:, 0:NL], in0=hh[:, 0:NL], in1=h_f[:, m, 0:NL])
                p = sm.tile([P, N], BF16, tag=f"p{m}", bufs=3, name="p")
                nc.vector.tensor_mul(out=p[:, 0:NL], in0=u[:, 0:NL], in1=d[:, 0:NL])
                q = sm.tile([P, N], BF16, tag=f"q{m}", bufs=3, name="q")
                nc.vector.tensor_mul(out=q[:, 0:NL], in0=a_bc[:, 0:NL], in1=p[:, 0:NL])
                nc.gpsimd.dma_start(out=o_d.ap()[ms, n0:n0 + NL], in_=q[:, 0:NL],
                                    accum_op=mybir.AluOpType.add)

    split_multi_waits(nc)
    return nc


def pack_consts(Wu, Uu, bu, Wr, Ur, br, Wh, Uh, bh):
    def wpack(w):
        return w.reshape(2, P, D).transpose(1, 0, 2).reshape(P, 2 * D)

    cols = []
    for wg, ug in ((Wu, Uu), (Wr, Ur), (Wh, Uh)):
        cols.append(wpack(np.asarray(wg, np.float32)))
        cols.append(wpack(np.asarray(ug, np.float32)))
    for bvec in (bu, br, bh):
        cols.append(np.asarray(bvec, np.float32).reshape(2, P).T)  # [p, 2]
    return np.ascontiguousarray(np.concatenate(cols, axis=1), np.float32)


def make_cbf():
    return np.ascontiguousarray(
        np.concatenate([np.eye(P), np.ones((P, 128))], axis=1)
    ).astype(ml_dtypes.bfloat16)


_CACHE = {}


def _get_nc(npass=1):
    if npass not in _CACHE:
        _CACHE[npass] = build(npass)
    return _CACHE[npass]


def run_spmd_prefill(nc, in_maps, out_prefill, n_cores):
    """Like bass2jax.run_bass_via_pjrt but the donated output buffers are
    prefilled with `out_prefill[name]` per core (the kernel accumulates onto
    outT, which must start as hT)."""
    import jax
    from jax.sharding import Mesh, PartitionSpec
    from jax.experimental.shard_map import shard_map as shard_map_fn

    b2j.install_neuronx_cc_hook()
    partition_name = nc.partition_id_tensor.name if nc.partition_id_tensor else None
    in_names, out_names, out_avals = [], [], []
    for alloc in nc.m.functions[0].allocations:
        if not isinstance(alloc, mybir.MemoryLocationSet):
            continue
        name = alloc.memorylocations[0].name
        if alloc.kind == "ExternalInput":
            if name != partition_name:
                in_names.append(name)
        elif alloc.kind == "ExternalOutput":
            out_names.append(name)
            out_avals.append(jax.core.ShapedArray(
                tuple(alloc.tensor_shape), mybir.dt.np(alloc.dtype)))
    n_params = len(in_names)
    all_in_names = in_names + out_names
    if partition_name is not None:
        all_in_names = all_in_names + [partition_name]
    donate = tuple(range(n_params, n_params + len(out_names)))

    def _body(*args):
        operands = list(args)
        if partition_name is not None:
            operands.append(b2j.partition_id_tensor())
        outs = b2j._bass_exec_p.bind(
            *operands,
            out_avals=tuple(out_avals), in_names=tuple(all_in_names),
            out_names=tuple(out_names), lowering_input_output_aliases=(),
            sim_require_finite=True, sim_require_nnan=True, nc=nc)
        return tuple(outs)

    devices = jax.devices()[:n_cores]
    mesh = Mesh(np.asarray(devices), ("core",))
    fn = jax.jit(
        shard_map_fn(_body, mesh=mesh,
                     in_specs=(PartitionSpec("core"),) * (n_params + len(out_names)),
                     out_specs=(PartitionSpec("core"),) * len(out_names),
                     check_rep=False),
        donate_argnums=donate, keep_unused=True)
    concat_in = [
        np.concatenate([np.asarray(in_maps[c][nm]) for c in range(n_cores)], axis=0)
        for nm in in_names]
    concat_fill = [
        np.concatenate([np.asarray(out_prefill[c][nm]) for c in range(n_cores)], axis=0)
        for nm in out_names]
    out_arrs = fn(*concat_in, *concat_fill)
    return [
        {nm: np.asarray(out_arrs[i]).reshape(n_cores, *out_avals[i].shape)[c]
         for i, nm in enumerate(out_names)}
        for c in range(n_cores)
    ]


def kernel(x, h_1, a, Wu, Uu, bu, Wr, Ur, br, Wh, Uh, bh):
    nc = _get_nc()
    consts = pack_consts(Wu, Uu, bu, Wr, Ur, br, Wh, Uh, bh)
    cbf = make_cbf()
    x = np.asarray(x, np.float32)
    h_1 = np.asarray(h_1, np.float32)
    a = np.asarray(a, np.float32)
    in_maps, prefills = [], []
    for c in range(NCORES):
        sl = slice(c * BL, (c + 1) * BL)
        hT = np.ascontiguousarray(h_1[sl].T)
        in_maps.append({
            "consts": consts,
            "cbf": cbf,
            "xT": np.ascontiguousarray(x[sl].T),
            "hT": hT,
            "aT": np.ascontiguousarray(a[sl].T).astype(ml_dtypes.bfloat16),
        })
        prefills.append({"outT": hT})
    results = run_spmd_prefill(nc, in_maps, prefills, NCORES)
    out = np.empty((B, D), np.float32)
    for c in range(NCORES):
        out[c * BL:(c + 1) * BL] = results[c]["outT"].T
    return out

